# revision 3
# baseline (speedup 1.0000x reference)
"""MoE FFN (8 experts, top-2 routing + shared expert) on 8 Trainium2 cores.

Strategy (expert-parallel, host-side dispatch):
- Host computes the router (softmax + top-2 + renorm) in numpy, gathers each
  expert's tokens into a padded [D, CAP] transposed batch (so the device
  needs no transposes), and splits tokens 8 ways for the shared expert.
- Core c runs a SwiGLU FFN over expert c's token batch with expert c's
  weights, and over its 1/8 token slice with the shared weights. All matmuls
  are float32r (full-rate fp32 PE mode), accumulated in fp32 PSUM.
- Host scales each expert's output rows by the renormalized top-2 probs and
  scatter-adds into the final [N, D] output along with the shared output.

Layouts on device: activations are [feature, token] so both FFN layers
contract along the partition dim with no transposes; outputs come back
transposed and the host untransposes.
"""
import sys

sys.path.insert(0, "/opt/trn_rl_repo")

import numpy as np

import concourse.bass as bass
import concourse.mybir as mybir
import concourse.tile as tile
from concourse.bass_utils import run_bass_kernel_spmd
from concourse.vector_clock import ScopedClock

D = 1024
F = 1024
E = 8
NCORES = 8
KTOP = 2
N_TOK = 8192  # 4 * 2048
NSH = N_TOK // NCORES  # shared-expert tokens per core
TB = 512  # token block (one fp32 PSUM bank)
KT = D // 128  # contraction tiles per matmul

F32 = mybir.dt.float32
F32R = mybir.dt.float32r
SILU = mybir.ActivationFunctionType.Silu


class _TileContext(tile.TileContext):
    """Walrus in this container rejects >1 sync wait per instruction; split
    the final multi-wait SP drain into a chain of single-wait drains."""

    def _drain_and_barrier(self, tick_clock, wait_clock):
        nc = self.nc
        drain_inst = nc.sync.drain()
        wait_clock.add_sem_waits(
            drain_inst.ins, ScopedClock({None: tick_clock.global_clock})
        )
        si = drain_inst.ins.sync_info
        waits = list(si.on_wait) if si is not None else []
        if len(waits) > 1:
            upd = list(si.on_update) if si is not None else []
            drain_inst.ins.sync_info = mybir.SyncInfo(
                on_wait=waits[:1], on_update=[]
            )
            for k, w in enumerate(waits[1:]):
                extra = nc.sync.drain()
                extra.ins.sync_info = mybir.SyncInfo(
                    on_wait=[w], on_update=upd if k == len(waits) - 2 else []
                )
        nc.all_engine_barrier()
        assert self.sems is not None
        popped = nc._tile_sem_poison_stack.pop()
        assert popped is self._sem_poison
        nc.clear_and_free_semaphores(list(self.sems.allocated().values()))
        nc.all_engine_barrier()


_ws_counter = [0]


def _split_multiwaits(nc):
    """Walrus here allows a single sync wait per instruction. Prefix each
    multi-wait instruction with single-wait NOPs on the same engine (engine
    queues run in order, so this is semantically identical)."""
    for f in nc.m.functions:
        for bb in f.blocks:
            insts = bb.instructions
            new_list = []
            changed = False
            for inst in insts:
                si = getattr(inst, "sync_info", None)
                waits = list(si.on_wait) if si is not None else []
                if len(waits) > 1:
                    changed = True
                    for w in waits[:-1]:
                        _ws_counter[0] += 1
                        nop = mybir.InstNoOp(name=f"I-waitsplit-{_ws_counter[0]}")
                        nop.engine = inst.engine
                        nop.sync_info = mybir.SyncInfo(on_wait=[w], on_update=[])
                        new_list.append(nop)
                    inst.sync_info = mybir.SyncInfo(
                        on_wait=[waits[-1]], on_update=list(si.on_update)
                    )
                new_list.append(inst)
            if changed:
                insts[:] = new_list
    return nc


def _emit_ffn(nc, wpool, apool, pspool, xdram, ncols, wg_d, wu_d, wd_d, odram):
    """SwiGLU FFN: odram[d, t] = (silu(x@wg) * (x@wu)) @ wd, all operands
    feature-major ([feature, token]); weights stream into resident SBUF tiles."""
    wgt, wut, wdt = [], [], []
    for k in range(KT):
        t = wpool.tile([128, F], F32R, tag=f"wg{k}", bufs=1)
        nc.sync.dma_start(t[:], wg_d[k * 128 : (k + 1) * 128, :])
        wgt.append(t)
    for k in range(KT):
        t = wpool.tile([128, F], F32R, tag=f"wu{k}", bufs=1)
        nc.sync.dma_start(t[:], wu_d[k * 128 : (k + 1) * 128, :])
        wut.append(t)
    for k in range(KT):
        t = wpool.tile([128, D], F32R, tag=f"wd{k}", bufs=1)
        nc.sync.dma_start(t[:], wd_d[k * 128 : (k + 1) * 128, :])
        wdt.append(t)

    for b in range(ncols // TB):
        cs = slice(b * TB, (b + 1) * TB)
        xts = []
        for k in range(KT):
            xt = apool.tile([128, TB], F32R, tag="xt", bufs=12)
            nc.sync.dma_start(xt[:], xdram[k * 128 : (k + 1) * 128, cs])
            xts.append(xt)
        hs = []
        for fj in range(F // 128):
            fs = slice(fj * 128, (fj + 1) * 128)
            pg = pspool.tile([128, TB], F32, tag="pg", bufs=2)
            pu = pspool.tile([128, TB], F32, tag="pu", bufs=2)
            for k in range(KT):
                nc.tensor.matmul(
                    pg[:], wgt[k][:, fs], xts[k][:], start=(k == 0), stop=(k == KT - 1)
                )
            for k in range(KT):
                nc.tensor.matmul(
                    pu[:], wut[k][:, fs], xts[k][:], start=(k == 0), stop=(k == KT - 1)
                )
            hg = apool.tile([128, TB], F32, tag="hg", bufs=3)
            nc.scalar.activation(hg[:], pg[:], SILU)
            h = apool.tile([128, TB], F32R, tag="h", bufs=12)
            nc.vector.tensor_mul(h[:], hg[:], pu[:])
            hs.append(h)
        for dj in range(D // 128):
            ds_ = slice(dj * 128, (dj + 1) * 128)
            po = pspool.tile([128, TB], F32, tag="po", bufs=3)
            for fk in range(F // 128):
                nc.tensor.matmul(
                    po[:], wdt[fk][:, ds_], hs[fk][:], start=(fk == 0),
                    stop=(fk == F // 128 - 1),
                )
            ot = apool.tile([128, TB], F32, tag="ot", bufs=4)
            nc.vector.tensor_copy(ot[:], po[:])
            nc.sync.dma_start(odram[ds_, cs], ot[:])


def build_program(cap, iters=None):
    """Build the per-core Bass program. `iters` wraps the body in a repeat
    loop (used only for benchmarking); None means single-shot."""
    nc = bass.Bass(num_devices=NCORES)
    xT = nc.dram_tensor("xT", [D, cap], F32R, kind="ExternalInput")
    xsT = nc.dram_tensor("xsT", [D, NSH], F32R, kind="ExternalInput")
    wg = nc.dram_tensor("wg", [D, F], F32R, kind="ExternalInput")
    wu = nc.dram_tensor("wu", [D, F], F32R, kind="ExternalInput")
    wd = nc.dram_tensor("wd", [F, D], F32R, kind="ExternalInput")
    sg = nc.dram_tensor("sg", [D, F], F32R, kind="ExternalInput")
    su = nc.dram_tensor("su", [D, F], F32R, kind="ExternalInput")
    sd = nc.dram_tensor("sd", [F, D], F32R, kind="ExternalInput")
    oT = nc.dram_tensor("oT", [D, cap], F32, kind="ExternalOutput")
    osT = nc.dram_tensor("osT", [D, NSH], F32, kind="ExternalOutput")

    with _TileContext(nc) as tc:
        with (
            tc.tile_pool(name="w", bufs=1) as wpool,
            tc.tile_pool(name="a", bufs=1) as apool,
            tc.tile_pool(name="ps", bufs=1, space="PSUM") as pspool,
        ):

            def body():
                _emit_ffn(nc, wpool, apool, pspool, xT, cap, wg, wu, wd, oT)
                _emit_ffn(nc, wpool, apool, pspool, xsT, NSH, sg, su, sd, osT)

            if iters is None:
                body()
            else:
                with tc.For_i(0, iters, 1):
                    body()
    _split_multiwaits(nc)
    return nc


def _route(flat, router_w):
    """Host router: softmax + top-2 + renormalize (matches reference)."""
    logits = flat @ router_w
    logits = logits - logits.max(axis=-1, keepdims=True)
    p = np.exp(logits)
    p /= p.sum(axis=-1, keepdims=True)
    idx = np.argpartition(p, -KTOP, axis=-1)[:, -KTOP:]  # [N, 2] (unordered)
    pv = np.take_along_axis(p, idx, axis=-1)
    pv = pv / (pv.sum(axis=-1, keepdims=True) + 1e-9)
    return idx, pv


def make_in_maps(x, router_w, sg, su, sd, wg, wu, wd):
    flat = np.ascontiguousarray(np.asarray(x, dtype=np.float32).reshape(-1, D))
    idx, pv = _route(flat, np.asarray(router_w, dtype=np.float32))

    tok_idx, tok_w = [], []
    for e in range(E):
        mask = idx == e  # [N, 2]; top-2 experts are distinct -> <=1 True per row
        tok_idx.append(np.nonzero(mask)[0])
        tok_w.append(pv[mask].astype(np.float32))
    max_cnt = max(len(r) for r in tok_idx)
    cap = max(TB, -(-max_cnt // TB) * TB)

    sgc = np.ascontiguousarray(np.asarray(sg, dtype=np.float32))
    suc = np.ascontiguousarray(np.asarray(su, dtype=np.float32))
    sdc = np.ascontiguousarray(np.asarray(sd, dtype=np.float32))
    in_maps = []
    for c in range(NCORES):
        rows = tok_idx[c]
        xTc = np.zeros((D, cap), dtype=np.float32)
        xTc[:, : len(rows)] = flat[rows].T
        xsTc = np.ascontiguousarray(flat[c * NSH : (c + 1) * NSH].T)
        in_maps.append(
            {
                "xT": xTc,
                "xsT": xsTc,
                "wg": np.ascontiguousarray(np.asarray(wg[c], dtype=np.float32)),
                "wu": np.ascontiguousarray(np.asarray(wu[c], dtype=np.float32)),
                "wd": np.ascontiguousarray(np.asarray(wd[c], dtype=np.float32)),
                "sg": sgc,
                "su": suc,
                "sd": sdc,
            }
        )
    return in_maps, tok_idx, tok_w, cap


def combine(results, tok_idx, tok_w, shape):
    out = np.empty((N_TOK, D), dtype=np.float32)
    for c in range(NCORES):
        out[c * NSH : (c + 1) * NSH] = results[c]["osT"].T
    for c in range(NCORES):
        rows = tok_idx[c]
        if len(rows):
            out[rows] += tok_w[c][:, None] * results[c]["oT"].T[: len(rows)]
    return out.reshape(shape)


def kernel(x, router_w, sg, su, sd, wg, wu, wd):
    x = np.asarray(x)
    in_maps, tok_idx, tok_w, cap = make_in_maps(
        x, router_w, sg, su, sd, wg, wu, wd
    )
    nc = build_program(cap)
    res = run_bass_kernel_spmd(nc, in_maps, core_ids=list(range(NCORES)))
    return combine(res.results, tok_idx, tok_w, x.shape)


# revision 6
# speedup vs baseline: 8.4393x; 8.4393x over previous
"""MoE FFN (8 experts, top-2 routing + shared expert) on 8 Trainium2 cores.

Strategy (expert-parallel, host-side dispatch):
- Host computes the router (softmax + top-2 + renorm) in numpy, gathers each
  expert's tokens into a padded [D, CAP] transposed batch (so the device
  needs no transposes), and splits tokens 8 ways for the shared expert.
- Core c runs a SwiGLU FFN over expert c's token batch with expert c's
  weights, and over its 1/8 token slice with the shared weights. All matmuls
  are float32r (full-rate fp32 PE mode), accumulated in fp32 PSUM.
- Host scales each expert's output rows by the renormalized top-2 probs and
  scatter-adds into the final [N, D] output along with the shared output.

Layouts on device: activations are [feature, token] so both FFN layers
contract along the partition dim with no transposes; outputs come back
transposed and the host untransposes.
"""
import sys

sys.path.insert(0, "/opt/trn_rl_repo")

import numpy as np

import concourse.bass as bass
import concourse.mybir as mybir
import concourse.tile as tile
from concourse.bass_utils import run_bass_kernel_spmd
from concourse.vector_clock import ScopedClock

D = 1024
F = 1024
E = 8
NCORES = 8
KTOP = 2
N_TOK = 8192  # 4 * 2048
NSH = N_TOK // NCORES  # shared-expert tokens per core
TB = 512  # token block (one fp32 PSUM bank)
KT = D // 128  # contraction tiles per matmul

F32 = mybir.dt.float32
F32R = mybir.dt.float32r
SILU = mybir.ActivationFunctionType.Silu


class _TileContext(tile.TileContext):
    """Walrus in this container rejects >1 sync wait per instruction; split
    the final multi-wait SP drain into a chain of single-wait drains."""

    def _drain_and_barrier(self, tick_clock, wait_clock):
        nc = self.nc
        drain_inst = nc.sync.drain()
        wait_clock.add_sem_waits(
            drain_inst.ins, ScopedClock({None: tick_clock.global_clock})
        )
        si = drain_inst.ins.sync_info
        waits = list(si.on_wait) if si is not None else []
        if len(waits) > 1:
            upd = list(si.on_update) if si is not None else []
            drain_inst.ins.sync_info = mybir.SyncInfo(
                on_wait=waits[:1], on_update=[]
            )
            for k, w in enumerate(waits[1:]):
                extra = nc.sync.drain()
                extra.ins.sync_info = mybir.SyncInfo(
                    on_wait=[w], on_update=upd if k == len(waits) - 2 else []
                )
        nc.all_engine_barrier()
        assert self.sems is not None
        popped = nc._tile_sem_poison_stack.pop()
        assert popped is self._sem_poison
        nc.clear_and_free_semaphores(list(self.sems.allocated().values()))
        nc.all_engine_barrier()


_ws_counter = [0]


def _split_multiwaits(nc):
    """Walrus here allows a single sync wait per instruction. Prefix each
    multi-wait instruction with single-wait NOPs on the same engine (engine
    queues run in order, so this is semantically identical)."""
    for f in nc.m.functions:
        for bb in f.blocks:
            insts = bb.instructions
            new_list = []
            changed = False
            for inst in insts:
                si = getattr(inst, "sync_info", None)
                waits = list(si.on_wait) if si is not None else []
                if len(waits) > 1:
                    changed = True
                    for w in waits[:-1]:
                        _ws_counter[0] += 1
                        nop = mybir.InstNoOp(name=f"I-waitsplit-{_ws_counter[0]}")
                        nop.engine = inst.engine
                        nop.sync_info = mybir.SyncInfo(on_wait=[w], on_update=[])
                        new_list.append(nop)
                    inst.sync_info = mybir.SyncInfo(
                        on_wait=[waits[-1]], on_update=list(si.on_update)
                    )
                new_list.append(inst)
            if changed:
                insts[:] = new_list
    return nc


def _col_blocks(ncols):
    """Split ncols (multiple of 128) into blocks of <=TB columns."""
    blocks, start = [], 0
    while start < ncols:
        w = min(TB, ncols - start)
        blocks.append((start, w))
        start += w
    return blocks


def _emit_ffn(
    nc, wpool, apool, pspool, xdram, ncols, wg_d, wu_d, wd_d, odram, tags
):
    """SwiGLU FFN: odram[d, t] = (silu(x@wg) * (x@wu)) @ wd, all operands
    feature-major ([feature, token]); weights stream into resident SBUF tiles.
    `tags` names the weight slot tags (reuse across phases = serialized load,
    fresh tag = prefetched during the previous phase)."""
    blocks = _col_blocks(ncols)
    gtag, utag, dtag = tags

    # first token block before the weights: PE needs x + the first wg k-tile
    # to start, and the weight stream is 12 MiB deep.
    xts0 = []
    for k in range(KT):
        xt = apool.tile([128, blocks[0][1]], F32R, tag="xt", bufs=12)
        nc.sync.dma_start(
            xt[:], xdram[k * 128 : (k + 1) * 128, 0 : blocks[0][1]]
        )
        xts0.append(xt)

    wgt, wut, wdt = [], [], []
    for k in range(KT):
        t = wpool.tile([128, F], F32R, tag=f"{gtag}{k}", bufs=1)
        nc.sync.dma_start(t[:], wg_d[k * 128 : (k + 1) * 128, :])
        wgt.append(t)
    for k in range(KT):
        t = wpool.tile([128, F], F32R, tag=f"{utag}{k}", bufs=1)
        nc.sync.dma_start(t[:], wu_d[k * 128 : (k + 1) * 128, :])
        wut.append(t)
    for k in range(KT):
        t = wpool.tile([128, D], F32R, tag=f"{dtag}{k}", bufs=1)
        nc.sync.dma_start(t[:], wd_d[k * 128 : (k + 1) * 128, :])
        wdt.append(t)

    for b, (c0, cw) in enumerate(blocks):
        cs = slice(c0, c0 + cw)
        if b == 0:
            xts = xts0
        else:
            xts = []
            for k in range(KT):
                xt = apool.tile([128, cw], F32R, tag="xt", bufs=12)
                nc.sync.dma_start(xt[:], xdram[k * 128 : (k + 1) * 128, cs])
                xts.append(xt)
        hs = []
        for fj in range(F // 128):
            fs = slice(fj * 128, (fj + 1) * 128)
            pg = pspool.tile([128, cw], F32, tag="pg", bufs=2)
            pu = pspool.tile([128, cw], F32, tag="pu", bufs=2)
            for k in range(KT):
                nc.tensor.matmul(
                    pg[:], wgt[k][:, fs], xts[k][:], start=(k == 0), stop=(k == KT - 1)
                )
            for k in range(KT):
                nc.tensor.matmul(
                    pu[:], wut[k][:, fs], xts[k][:], start=(k == 0), stop=(k == KT - 1)
                )
            hg = apool.tile([128, cw], F32, tag="hg", bufs=3)
            nc.scalar.activation(hg[:], pg[:], SILU)
            h = apool.tile([128, cw], F32R, tag="h", bufs=12)
            nc.vector.tensor_mul(h[:], hg[:], pu[:])
            hs.append(h)
        for dj in range(D // 128):
            ds_ = slice(dj * 128, (dj + 1) * 128)
            po = pspool.tile([128, cw], F32, tag="po", bufs=3)
            for fk in range(F // 128):
                nc.tensor.matmul(
                    po[:], wdt[fk][:, ds_], hs[fk][:], start=(fk == 0),
                    stop=(fk == F // 128 - 1),
                )
            ot = apool.tile([128, cw], F32, tag="ot", bufs=4)
            nc.vector.tensor_copy(ot[:], po[:])
            nc.sync.dma_start(odram[ds_, cs], ot[:])


def build_program(cap, iters=None):
    """Build the per-core Bass program. `iters` wraps the body in a repeat
    loop (used only for benchmarking); None means single-shot."""
    nc = bass.Bass(num_devices=NCORES)
    xT = nc.dram_tensor("xT", [D, cap], F32R, kind="ExternalInput")
    xsT = nc.dram_tensor("xsT", [D, NSH], F32R, kind="ExternalInput")
    wg = nc.dram_tensor("wg", [D, F], F32R, kind="ExternalInput")
    wu = nc.dram_tensor("wu", [D, F], F32R, kind="ExternalInput")
    wd = nc.dram_tensor("wd", [F, D], F32R, kind="ExternalInput")
    sg = nc.dram_tensor("sg", [D, F], F32R, kind="ExternalInput")
    su = nc.dram_tensor("su", [D, F], F32R, kind="ExternalInput")
    sd = nc.dram_tensor("sd", [F, D], F32R, kind="ExternalInput")
    oT = nc.dram_tensor("oT", [D, cap], F32, kind="ExternalOutput")
    osT = nc.dram_tensor("osT", [D, NSH], F32, kind="ExternalOutput")

    with _TileContext(nc) as tc:
        with (
            tc.tile_pool(name="w", bufs=1) as wpool,
            tc.tile_pool(name="a", bufs=1) as apool,
            tc.tile_pool(name="ps", bufs=1, space="PSUM") as pspool,
        ):

            def body():
                # shared gate weights get dedicated slots ("sg" tags) so
                # their DMA prefetches during the routed phase; su/sd reuse
                # the wu/wd slots (loads overlap the routed tail).
                _emit_ffn(
                    nc, wpool, apool, pspool, xT, cap, wg, wu, wd, oT,
                    ("wg", "wu", "wd"),
                )
                _emit_ffn(
                    nc, wpool, apool, pspool, xsT, NSH, sg, su, sd, osT,
                    ("sg", "wu", "wd"),
                )

            if iters is None:
                body()
            else:
                with tc.For_i(0, iters, 1):
                    body()
    _split_multiwaits(nc)
    return nc


def _route(flat, router_w):
    """Host router: softmax + top-2 + renormalize (matches reference)."""
    logits = flat @ router_w
    logits = logits - logits.max(axis=-1, keepdims=True)
    p = np.exp(logits)
    p /= p.sum(axis=-1, keepdims=True)
    idx = np.argpartition(p, -KTOP, axis=-1)[:, -KTOP:]  # [N, 2] (unordered)
    pv = np.take_along_axis(p, idx, axis=-1)
    pv = pv / (pv.sum(axis=-1, keepdims=True) + 1e-9)
    return idx, pv


def make_in_maps(x, router_w, sg, su, sd, wg, wu, wd):
    flat = np.ascontiguousarray(np.asarray(x, dtype=np.float32).reshape(-1, D))
    idx, pv = _route(flat, np.asarray(router_w, dtype=np.float32))

    tok_idx, tok_w = [], []
    for e in range(E):
        mask = idx == e  # [N, 2]; top-2 experts are distinct -> <=1 True per row
        tok_idx.append(np.nonzero(mask)[0])
        tok_w.append(pv[mask].astype(np.float32))
    max_cnt = max(len(r) for r in tok_idx)
    cap = max(TB, -(-max_cnt // 128) * 128)

    sgc = np.ascontiguousarray(np.asarray(sg, dtype=np.float32))
    suc = np.ascontiguousarray(np.asarray(su, dtype=np.float32))
    sdc = np.ascontiguousarray(np.asarray(sd, dtype=np.float32))
    in_maps = []
    for c in range(NCORES):
        rows = tok_idx[c]
        xTc = np.zeros((D, cap), dtype=np.float32)
        xTc[:, : len(rows)] = flat[rows].T
        xsTc = np.ascontiguousarray(flat[c * NSH : (c + 1) * NSH].T)
        in_maps.append(
            {
                "xT": xTc,
                "xsT": xsTc,
                "wg": np.ascontiguousarray(np.asarray(wg[c], dtype=np.float32)),
                "wu": np.ascontiguousarray(np.asarray(wu[c], dtype=np.float32)),
                "wd": np.ascontiguousarray(np.asarray(wd[c], dtype=np.float32)),
                "sg": sgc,
                "su": suc,
                "sd": sdc,
            }
        )
    return in_maps, tok_idx, tok_w, cap


def combine(results, tok_idx, tok_w, shape):
    out = np.empty((N_TOK, D), dtype=np.float32)
    for c in range(NCORES):
        out[c * NSH : (c + 1) * NSH] = results[c]["osT"].T
    for c in range(NCORES):
        rows = tok_idx[c]
        if len(rows):
            out[rows] += tok_w[c][:, None] * results[c]["oT"].T[: len(rows)]
    return out.reshape(shape)


def kernel(x, router_w, sg, su, sd, wg, wu, wd):
    x = np.asarray(x)
    in_maps, tok_idx, tok_w, cap = make_in_maps(
        x, router_w, sg, su, sd, wg, wu, wd
    )
    nc = build_program(cap)
    res = run_bass_kernel_spmd(nc, in_maps, core_ids=list(range(NCORES)))
    return combine(res.results, tok_idx, tok_w, x.shape)
